# revision 42
# baseline (speedup 1.0000x reference)
"""Causal single-head attention on 8 trn2 NeuronCores — fp8 DoubleRow version.

Sharding: core c handles batch c//2 and the query rows congruent to h = c%2
(mod 2). The mod-2 interleave makes the causal work of the two cores of a
batch identical tile-by-tile, so one uniform device program wastes nothing:
slot s (128 queries, global rows 2*(128s+i)+h) attends exactly 2s+2 key
tiles, and only the last two need masks (the same two masks for every slot).

Algorithm — projections folded, all matmuls fp8 e4m3 DoubleRow (2 packed
k-tiles per instruction at 0.5 cycles/row):
  host:  A32 = 32 * Wq^T Wk, Wv32 = 32 * Wv^T   (scaled so fp8 hi/lo splits
         stay out of e4m3's subnormal range), x split to (hi, lo) e4m3 pairs
         in the DoubleRow layouts (contraction pairs packed on the free dim).
  dev:   T^T = A32h^T xh^T + A32h^T xl^T[first half of d1]   (own queries)
         Th  = fp8(T psum)
         S^T = xh^T.T Th^T  (per 128-query slot) -> +mask, e = exp(S/1024)
         probs = (ph, pl) e4m3                    (hi copy DVE, residual
                                                   GPSIMD)
         U^T  = (xh/2 + xl/2)^T ph + (xh/2)^T pl -> (Uh, Ul) e4m3
         l16  = ones16^T (ph + pl)                (DoubleRow, 1-col)
         out  = ((Uh+Ul)^T.T Wv32h^T + Uh^T.T Wv32l^T) * recip(l16) -> bf16.
  The value path (U, out) keeps full hi/lo 3-combo accuracy; the softmax
  path drops selected lo cross terms (score errors are strongly attenuated
  through the normalized softmax), landing at rel err ~1.6e-2 vs the 2e-2
  gate.

Scheduling: every PSUM accumulation window is a full 2KB bank ([128,2,256]
f32) shared by 2-4 logical 128/256-wide groups under one start/stop
(pending-zero zeroes the whole bank at start), cutting quantization ops in
half and doubling ring depth; one 7-bank ring serves T/scores/U/out windows
in emission order. A round-robin zipper interleaves scores(s), U(s-2),
out(s-5) windows for continuous PE feed; DMAs are merged per array (HWDGE
cost) and chunked where consumers are partial (xq per query block, xk per
key half, xn per key quad, T tiles per query block).
"""

import sys

try:
    import concourse  # noqa: F401
except ImportError:
    sys.path.insert(0, "/opt/trn_rl_repo")

from contextlib import ExitStack

import ml_dtypes
import numpy as np

import concourse.bass as bass
from concourse import bacc
import concourse.mybir as mybir
import concourse.tile as tile
from concourse.bass_utils import run_bass_kernel_spmd

B, N, D = 4, 2048, 1024
NQ = 1024
NCORES = 8
NS = 8                   # query slots per core
IBQ = 128                # queries per slot
IB = 256
SCALE = 1.0 / 1024.0     # exp scale: 1/sqrt(D) / 32 (A32 carries 32x)
P = 128

F8 = mybir.dt.float8e4
F32 = mybir.dt.float32
BF16 = mybir.dt.bfloat16
DRM = mybir.MatmulPerfMode.DoubleRow
E4NP = ml_dtypes.float8_e4m3
BFNP = ml_dtypes.bfloat16

TRACE = False
LAST_RESULT = None
LAST_IN_MAPS = None
_CACHED_NC = None


def _qrows(h):
    return np.arange(h, N, 2)


def _trips(s):
    return 2 * s + 2


def _build_masks(h):
    """[2, 128, 128] f32 additive masks for the two diagonal key tiles of
    every slot: key (2s+k)*128+jp vs query row 2*(128s+iv)+h."""
    jp = np.arange(P)[:, None]
    iv = np.arange(IBQ)[None, :]
    m0 = np.where(jp <= 2 * iv + h, 0.0, -1e30)
    m1 = np.where(128 + jp <= 2 * iv + h, 0.0, -1e30)
    return np.stack([m0, m1]).astype(np.float32)


def _mm(nc, out, lhsT, rhs, start, stop):
    nc.tensor.matmul(out, lhsT=lhsT, rhs=rhs, start=start, stop=stop,
                     perf_mode=DRM)


def _build_body(nc, tc, ctx, dram):
    (a_d, xq_d, xk_d, xn_d, wv_d, mask_d, out_d) = dram

    pool_a = ctx.enter_context(tc.tile_pool(name="a", bufs=4))
    pool_xq = ctx.enter_context(tc.tile_pool(name="xq", bufs=8))
    pool_xk = ctx.enter_context(tc.tile_pool(name="xk", bufs=4))
    pool_xn = ctx.enter_context(tc.tile_pool(name="xn", bufs=4))
    pool_wv = ctx.enter_context(tc.tile_pool(name="wv", bufs=2))
    pool_mask = ctx.enter_context(tc.tile_pool(name="mask", bufs=1))
    pool_t = ctx.enter_context(tc.tile_pool(name="tq", bufs=16))
    pool_e = ctx.enter_context(tc.tile_pool(name="e32", bufs=6))
    pool_p = ctx.enter_context(tc.tile_pool(name="probs", bufs=24))
    pool_u = ctx.enter_context(tc.tile_pool(name="uq", bufs=12))
    pool_ob = ctx.enter_context(tc.tile_pool(name="ob", bufs=4))
    pool_rt = ctx.enter_context(tc.tile_pool(name="rt", bufs=6))
    pool_one = ctx.enter_context(tc.tile_pool(name="one", bufs=1))

    at = [None] * 2                        # [pphalf] -> [128, 2, 2, 8, 128]
    xqt = [[None] * 4 for _ in range(2)]   # [hl][qb] -> [128, 4 pp, 2, 256]
    xkt = [None] * 2                       # [khalf] -> [128, 4, 2, 1024]
    xnq = [[None] * 2 for _ in range(2)]   # [hl][quad] -> [128,4,2,8,128]
    wvt = [None] * 2                       # [hl] -> [128, 4, 2, 1024]

    def load_a(half):
        t = pool_a.tile([P, 2, 2, 8, P], F8, tag="a", name=f"a{half}")
        nc.sync.dma_start(out=t, in_=a_d[:, 2 * half:2 * half + 2, :, :, :])
        at[half] = t

    def load_xq(hl, qb):
        t = pool_xq.tile([P, 4, 2, IB], F8, tag="xq", name=f"xq{hl}_{qb}")
        nc.sync.dma_start(out=t, in_=xq_d[hl, qb, :, :, :, :])
        xqt[hl][qb] = t

    def load_xk(kh):
        t = pool_xk.tile([P, 4, 2, N // 2], F8, tag="xk", name=f"xk{kh}")
        nc.sync.dma_start(out=t, in_=xk_d[:, :, :,
                                          kh * (N // 2):(kh + 1) * (N // 2)])
        xkt[kh] = t

    def load_xn(hl, q):
        t = pool_xn.tile([P, 4, 2, 8, P], F8, tag="xn", name=f"xn{hl}_{q}")
        nc.sync.dma_start(out=t, in_=xn_d[hl, q, :, :, :, :, :])
        xnq[hl][q] = t

    def load_wv(hl):
        t = pool_wv.tile([P, 4, 2, NQ], F8, tag="wv", name=f"wv{hl}")
        nc.sync.dma_start(out=t, in_=wv_d[hl, :, :, :, :])
        wvt[hl] = t

    load_a(0)
    load_xq(0, 0)
    load_a(1)
    load_xq(1, 0)
    for qb in range(1, 4):
        load_xq(0, qb)
        load_xq(1, qb)
    load_xk(0)
    maskt = pool_mask.tile([P, 2, IBQ], BF16, tag="mask", name="maskt")
    nc.sync.dma_start(out=maskt, in_=mask_d[:, :, :])
    load_xn(0, 0)
    load_xn(1, 0)
    load_xk(1)
    load_wv(0)
    load_wv(1)
    load_xn(0, 1)
    load_xn(1, 1)
    ones = pool_one.tile([P, 2, 1], F8, tag="one", name="ones")
    nc.vector.memset(ones, 16.0)

    # T^T tiles per (qb, dp): [128, 2, 256] (dim1 = d2-tile parity)
    tht = [[pool_t.tile([P, 2, IB], F8, tag="tq", name=f"th{qb}_{dp}")
            for dp in range(4)] for qb in range(4)]

    with (
        tc.tile_pool(name="psA", bufs=7, space="PSUM") as ps_a,
        tc.tile_pool(name="psl", bufs=1, space="PSUM") as ps_l,
    ):
        probs = [None] * NS   # per slot: (ph quads, pl quads)
        rts = [None] * NS
        uqs = [None] * NS     # per slot: (uh quads, ul quads) by dt-half

        def emit_t(qb):
            # one full-bank window per d2-pair: halves are the two d2 tiles
            for dp in range(4):
                psw = ps_a.tile([P, 2, IB], F32, tag="ps", name=f"pt{qb}_{dp}")
                # x-lo contraction only over the first half of d1 (the
                # dropped Ah*xl tail contributes ~0.5% which the error
                # budget absorbs)
                k = 0
                for half in range(2):
                    d2t = 2 * dp + half
                    for hx in (0, 1):
                        for pp in range(4 if hx == 0 else 2):
                            _mm(nc, psw[:, half, :],
                                at[pp // 2][:, pp % 2, :, d2t, :],
                                xqt[hx][qb][:, pp, :, :],
                                start=(k == 0), stop=(k == 11))
                            k += 1
                nc.scalar.activation(tht[qb][dp], psw,
                                     mybir.ActivationFunctionType.Copy)

        def gen_scores(s):
            trips = _trips(s)
            qb, sh = s // 2, s % 2
            nw = (trips + 3) // 4
            ph_t = [pool_p.tile([P, 4, IBQ], F8, tag="probs",
                                name=f"ph{s}_{w}") for w in range(nw)]
            pl_t = [pool_p.tile([P, 4, IBQ], F8, tag="probs",
                                name=f"pl{s}_{w}") for w in range(nw)]
            probs[s] = (ph_t, pl_t)
            for w in range(nw):
                njt = min(4, trips - 4 * w)
                psw = ps_a.tile([P, 2, IB], F32, tag="ps", name=f"pss{s}_{w}")
                k = 0
                last = 8 * njt - 1
                for q in range(njt):
                    jt = 4 * w + q
                    for dp in range(4):
                        _mm(nc, psw[:, q // 2, (q % 2) * IBQ:(q % 2 + 1) * IBQ],
                            xkt[jt // 8][:, dp, :,
                                         (jt % 8) * P:(jt % 8 + 1) * P],
                            tht[qb][dp][:, :, sh * IBQ:(sh + 1) * IBQ],
                            start=(k == 0), stop=(k == last))
                        k += 1
                if w == nw - 1:
                    # mask the last two key tiles (the slot diagonal)
                    if njt == 2:
                        nc.vector.tensor_add(psw[:, 0, :], psw[:, 0, :],
                                             maskt)
                    else:
                        nc.vector.tensor_add(psw[:, 1, :], psw[:, 1, :],
                                             maskt)
                e = pool_e.tile([P, 2, IB], F32, tag="e32", name=f"e{s}_{w}")
                if njt == 4:
                    nc.scalar.activation(e, psw,
                                         mybir.ActivationFunctionType.Exp,
                                         scale=SCALE)
                    nc.vector.tensor_copy(ph_t[w], e)
                    nc.gpsimd.tensor_sub(pl_t[w], e, ph_t[w])
                else:
                    nc.scalar.activation(e[:, 0, :], psw[:, 0, :],
                                         mybir.ActivationFunctionType.Exp,
                                         scale=SCALE)
                    nc.vector.tensor_copy(ph_t[w][:, :2, :], e[:, 0, :])
                    nc.gpsimd.tensor_sub(pl_t[w][:, :2, :], e[:, 0, :],
                                         ph_t[w][:, :2, :])
                yield

        def gen_u(s):
            pairs = _trips(s) // 2
            ph_t, pl_t = probs[s]
            uh_t = [pool_u.tile([P, 4, IBQ], F8, tag="uq",
                                name=f"uh{s}_{h}") for h in range(2)]
            ul_t = [pool_u.tile([P, 4, IBQ], F8, tag="uq",
                                name=f"ul{s}_{h}") for h in range(2)]
            uqs[s] = (uh_t, ul_t)
            for h2 in range(2):
                psw = ps_a.tile([P, 2, IB], F32, tag="ps", name=f"psu{s}_{h2}")
                k = 0
                last = 12 * pairs - 1
                for q in range(4):
                    dt = 4 * h2 + q
                    for hx, hp in ((0, 0), (1, 0), (0, 1)):
                        pt_ = ph_t if hp == 0 else pl_t
                        for j in range(pairs):
                            _mm(nc, psw[:, q // 2, (q % 2) * IBQ:(q % 2 + 1) * IBQ],
                                xnq[hx][j // 4][:, j % 4, :, dt, :],
                                pt_[j // 2][:, 2 * (j % 2):2 * (j % 2) + 2, :],
                                start=(k == 0), stop=(k == last))
                            k += 1
                nc.scalar.activation(uh_t[h2], psw,
                                     mybir.ActivationFunctionType.Copy)
                nc.vector.tensor_sub(ul_t[h2], psw, uh_t[h2])
                yield
            # softmax denominator (x16): 1-col DoubleRow matmuls
            psl = ps_l.tile([P, 1], F32, tag="psl", name=f"psl{s}")
            k = 0
            last = 2 * pairs - 1
            for j in range(pairs):
                for t in (ph_t, pl_t):
                    _mm(nc, psl,
                        t[j // 2][:, 2 * (j % 2):2 * (j % 2) + 2, :], ones,
                        start=(k == 0), stop=(k == last))
                    k += 1
            rt = pool_rt.tile([P, 1], F32, tag="rt", name=f"rt{s}")
            nc.vector.reciprocal(rt, psl)
            rts[s] = rt
            yield

        def gen_out(s):
            uh_t, ul_t = uqs[s]
            ob_t = pool_ob.tile([P, D], BF16, tag="ob", name=f"ob{s}")
            for w in range(2):
                psw = ps_a.tile([P, 2, IB], F32, tag="ps", name=f"psf{s}_{w}")
                k = 0
                for half in range(2):
                    ob = 2 * w + half
                    for hu, hw in ((0, 0), (0, 1), (1, 0)):
                        ut = uh_t if hu == 0 else ul_t
                        for dp in range(4):
                            _mm(nc, psw[:, half, :],
                                ut[dp // 2][:, 2 * (dp % 2):2 * (dp % 2) + 2, :],
                                wvt[hw][:, dp, :, ob * IB:(ob + 1) * IB],
                                start=(k == 0), stop=(k == 23))
                            k += 1
                nc.vector.tensor_scalar_mul(ob_t[:, w * 2 * IB:(w + 1) * 2 * IB],
                                            psw, rts[s])
                if s == NS - 1 and w == 1:
                    nc.sync.dma_start(out=out_d[s * IBQ:(s + 1) * IBQ,
                                                :2 * IB],
                                      in_=ob_t[:, :2 * IB])
                    nc.sync.dma_start(out=out_d[s * IBQ:(s + 1) * IBQ,
                                                2 * IB:],
                                      in_=ob_t[:, 2 * IB:])
                yield
            if s != NS - 1:
                nc.sync.dma_start(out=out_d[s * IBQ:(s + 1) * IBQ, :],
                                  in_=ob_t)
            yield

        def rr(*gens):
            live = list(gens)
            while live:
                nxt = []
                for g in live:
                    try:
                        next(g)
                        nxt.append(g)
                    except StopIteration:
                        pass
                live = nxt

        emit_t(0)
        emit_t(1)
        emit_t(2)
        emit_t(3)
        rr(gen_scores(0))
        rr(gen_scores(1))
        rr(gen_scores(2), gen_u(0))
        rr(gen_scores(3), gen_u(1))
        rr(gen_scores(4), gen_u(2), gen_out(0))
        rr(gen_scores(5), gen_u(3), gen_out(1))
        rr(gen_scores(6), gen_u(4), gen_out(2))
        rr(gen_scores(7), gen_u(5), gen_out(3))
        rr(gen_u(6), gen_out(4))
        rr(gen_u(7), gen_out(5))
        rr(gen_out(6))
        rr(gen_out(7))


def _build_nc():
    nc = bacc.Bacc(None, target_bir_lowering=False)
    a_d = nc.declare_dram_parameter("a8", [P, 4, 2, 8, P], F8,
                                    isOutput=False)
    xq_d = nc.declare_dram_parameter("xq8", [2, 4, P, 4, 2, IB], F8,
                                     isOutput=False)
    xk_d = nc.declare_dram_parameter("xk8", [P, 4, 2, N], F8,
                                     isOutput=False)
    xn_d = nc.declare_dram_parameter("xn8", [2, 2, P, 4, 2, 8, P], F8,
                                    isOutput=False)
    wv_d = nc.declare_dram_parameter("wv8", [2, P, 4, 2, NQ], F8,
                                     isOutput=False)
    mask_d = nc.declare_dram_parameter("masks", [P, 2, IBQ], BF16,
                                       isOutput=False)
    out_d = nc.declare_dram_parameter("out_p", [NQ, D], BF16, isOutput=True)
    dram = (a_d, xq_d, xk_d, xn_d, wv_d, mask_d, out_d)

    with tile.TileContext(nc) as tc:
        with ExitStack() as ctx:
            _build_body(nc, tc, ctx, dram)
    nc.finalize()
    return nc


def _split8(a):
    h = a.astype(E4NP)
    l = (a - h.astype(np.float32)).astype(E4NP)
    return h, l


def _make_in_maps(x, W_q, W_k, W_v):
    wq = np.asarray(W_q, np.float32)
    wk = np.asarray(W_k, np.float32)
    wv = np.asarray(W_v, np.float32)
    A32 = (wq.T.astype(np.float64) @ wk.astype(np.float64) * 32.0).astype(np.float32)
    Wvt32 = np.ascontiguousarray(wv.T) * 32.0

    # a8: [2, 128, 4 pp, 2, 8, 128]
    def lhs_pack(m):       # [d1, d2] -> [128, 4, 2, 8, 128]
        return m.reshape(4, 2, P, 8, P).transpose(2, 0, 1, 3, 4)
    ah, _ = _split8(A32)
    a8 = np.ascontiguousarray(lhs_pack(ah))
    # wv8: [2, 128, 4 dp, 2, 1024]
    def rhs_pack(m):       # [d, o] -> [128, 4, 2, 1024]
        return m.reshape(4, 2, P, NQ).transpose(2, 0, 1, 3)
    wh, wl = _split8(Wvt32)
    wv8 = np.ascontiguousarray(np.stack([rhs_pack(wh), rhs_pack(wl)]))

    masks = [None, None]
    for h in range(2):
        mk = _build_masks(h).astype(BFNP)       # [2, 128, 128]
        masks[h] = np.ascontiguousarray(mk.transpose(1, 0, 2))
    qrows = [_qrows(0), _qrows(1)]

    in_maps = []
    per_batch = {}
    for b in range(B):
        xb = np.asarray(x[b], np.float32)
        xh, xl = _split8(xb)                    # [2048, 1024] e4m3
        xh32 = xh.astype(np.float32)
        xl32 = xl.astype(np.float32)
        # xk8: [2, 128, 4 dp, 2, 2048]: from x^T [1024 d, 2048 k]
        def xk_pack(m32):
            return m32.T.reshape(4, 2, P, N).transpose(2, 0, 1, 3)
        xk8 = np.ascontiguousarray(xk_pack(xh32)).astype(E4NP)
        # xn8: [2, 2 quad, 128, 4 kp, 2, 8 dt, 128]: from x/2 [2048, 1024]
        def xn_pack(m32):
            a = (m32 * 0.5).reshape(2, 4, 2, P, 8, P)
            return a.transpose(0, 3, 1, 2, 4, 5)
        xn8 = np.ascontiguousarray(np.stack(
            [xn_pack(xh32), xn_pack(xl32)])).astype(E4NP)
        per_batch[b] = (xh32, xl32, xk8, xn8)

    for c in range(NCORES):
        b, h = c // 2, c % 2
        xh32, xl32, xk8, xn8 = per_batch[b]
        # xq8: [2, 4 qb, 128, 4 pp, 2, 256]: x^T[:, qrows]
        def xq_pack(m32):
            # [1024 d1, 1024 q] -> [4 qb, 128, 4 pp, 2, 256]
            a = m32.T[:, qrows[h]].reshape(4, 2, P, 4, IB)
            return a.transpose(3, 2, 0, 1, 4)
        xq8 = np.ascontiguousarray(np.stack(
            [xq_pack(xh32), xq_pack(xl32)])).astype(E4NP)
        in_maps.append({
            "a8": a8, "xq8": xq8, "xk8": xk8, "xn8": xn8,
            "wv8": wv8, "masks": masks[h],
        })
    return in_maps


def kernel(x, W_q, W_k, W_v):
    global _CACHED_NC, LAST_RESULT, LAST_IN_MAPS
    x = np.asarray(x, dtype=np.float32)
    if _CACHED_NC is None:
        _CACHED_NC = _build_nc()
    nc = _CACHED_NC

    in_maps = _make_in_maps(x, W_q, W_k, W_v)
    LAST_IN_MAPS = in_maps
    try:
        res = run_bass_kernel_spmd(nc, in_maps, list(range(NCORES)))
    except Exception:
        # transient NRT_EXEC_UNIT_UNRECOVERABLE wedges clear on retry
        import time as _time
        _time.sleep(5)
        res = run_bass_kernel_spmd(nc, in_maps, list(range(NCORES)))
    LAST_RESULT = res

    qrows = [_qrows(0), _qrows(1)]
    out = np.empty((B, N, D), np.float32)
    for c in range(NCORES):
        b, h = c // 2, c % 2
        out[b, qrows[h], :] = np.asarray(res.results[c]["out_p"],
                                         dtype=np.float32)
    return out


# revision 43
# speedup vs baseline: 1.0071x; 1.0071x over previous
"""Causal single-head attention on 8 trn2 NeuronCores — fp8 DoubleRow version.

Sharding: core c handles batch c//2 and the query rows congruent to h = c%2
(mod 2). The mod-2 interleave makes the causal work of the two cores of a
batch identical tile-by-tile, so one uniform device program wastes nothing:
slot s (128 queries, global rows 2*(128s+i)+h) attends exactly 2s+2 key
tiles, and only the last two need masks (the same two masks for every slot).

Algorithm — projections folded, all matmuls fp8 e4m3 DoubleRow (2 packed
k-tiles per instruction at 0.5 cycles/row):
  host:  A32 = 32 * Wq^T Wk, Wv32 = 32 * Wv^T   (scaled so fp8 hi/lo splits
         stay out of e4m3's subnormal range), x split to (hi, lo) e4m3 pairs
         in the DoubleRow layouts (contraction pairs packed on the free dim).
  dev:   T^T = A32h^T xh^T + A32h^T xl^T[first half of d1]   (own queries)
         Th  = fp8(T psum)
         S^T = xh^T.T Th^T  (per 128-query slot) -> +mask, e = exp(S/1024)
         probs = (ph, pl) e4m3                    (hi copy DVE, residual
                                                   GPSIMD)
         U^T  = (xh/2 + xl/2)^T ph + (xh/2)^T pl -> (Uh, Ul) e4m3
         l16  = ones16^T (ph + pl)                (DoubleRow, 1-col)
         out  = ((Uh+Ul)^T.T Wv32h^T + Uh^T.T Wv32l^T) * recip(l16) -> bf16.
  The value path (U, out) keeps full hi/lo 3-combo accuracy; the softmax
  path drops selected lo cross terms (score errors are strongly attenuated
  through the normalized softmax), landing at rel err ~1.6e-2 vs the 2e-2
  gate.

Scheduling: every PSUM accumulation window is a full 2KB bank ([128,2,256]
f32) shared by 2-4 logical 128/256-wide groups under one start/stop
(pending-zero zeroes the whole bank at start), cutting quantization ops in
half and doubling ring depth; one 7-bank ring serves T/scores/U/out windows
in emission order. A round-robin zipper interleaves scores(s), U(s-2),
out(s-5) windows for continuous PE feed; DMAs are merged per array (HWDGE
cost) and chunked where consumers are partial (xq per query block, xk per
key half, xn per key quad, T tiles per query block).
"""

import sys

try:
    import concourse  # noqa: F401
except ImportError:
    sys.path.insert(0, "/opt/trn_rl_repo")

from contextlib import ExitStack

import ml_dtypes
import numpy as np

import concourse.bass as bass
from concourse import bacc
import concourse.mybir as mybir
import concourse.tile as tile
from concourse.bass_utils import run_bass_kernel_spmd

B, N, D = 4, 2048, 1024
NQ = 1024
NCORES = 8
NS = 8                   # query slots per core
IBQ = 128                # queries per slot
IB = 256
SCALE = 1.0 / 1024.0     # exp scale: 1/sqrt(D) / 32 (A32 carries 32x)
P = 128

F8 = mybir.dt.float8e4
F32 = mybir.dt.float32
BF16 = mybir.dt.bfloat16
DRM = mybir.MatmulPerfMode.DoubleRow
E4NP = ml_dtypes.float8_e4m3
BFNP = ml_dtypes.bfloat16

TRACE = False
LAST_RESULT = None
LAST_IN_MAPS = None
_CACHED_NC = None


def _qrows(h):
    return np.arange(h, N, 2)


def _trips(s):
    return 2 * s + 2


def _build_masks(h):
    """[2, 128, 128] f32 additive masks for the two diagonal key tiles of
    every slot: key (2s+k)*128+jp vs query row 2*(128s+iv)+h."""
    jp = np.arange(P)[:, None]
    iv = np.arange(IBQ)[None, :]
    m0 = np.where(jp <= 2 * iv + h, 0.0, -1e30)
    m1 = np.where(128 + jp <= 2 * iv + h, 0.0, -1e30)
    return np.stack([m0, m1]).astype(np.float32)


def _mm(nc, out, lhsT, rhs, start, stop):
    nc.tensor.matmul(out, lhsT=lhsT, rhs=rhs, start=start, stop=stop,
                     perf_mode=DRM)


def _build_body(nc, tc, ctx, dram):
    (a_d, xq_d, xk_d, xn_d, wv_d, mask_d, out_d) = dram

    pool_a = ctx.enter_context(tc.tile_pool(name="a", bufs=4))
    pool_xq = ctx.enter_context(tc.tile_pool(name="xq", bufs=8))
    pool_xk = ctx.enter_context(tc.tile_pool(name="xk", bufs=4))
    pool_xn = ctx.enter_context(tc.tile_pool(name="xn", bufs=4))
    pool_wv = ctx.enter_context(tc.tile_pool(name="wv", bufs=2))
    pool_mask = ctx.enter_context(tc.tile_pool(name="mask", bufs=1))
    pool_t = ctx.enter_context(tc.tile_pool(name="tq", bufs=16))
    pool_e = ctx.enter_context(tc.tile_pool(name="e32", bufs=6))
    pool_p = ctx.enter_context(tc.tile_pool(name="probs", bufs=24))
    pool_u = ctx.enter_context(tc.tile_pool(name="uq", bufs=12))
    pool_ob = ctx.enter_context(tc.tile_pool(name="ob", bufs=4))
    pool_rt = ctx.enter_context(tc.tile_pool(name="rt", bufs=6))
    pool_one = ctx.enter_context(tc.tile_pool(name="one", bufs=2))

    at = [None] * 2                        # [pphalf] -> [128, 2, 2, 8, 128]
    xqt = [[None] * 4 for _ in range(2)]   # [hl][qb] -> [128, 4 pp, 2, 256]
    xkt = [None] * 2                       # [khalf] -> [128, 4, 2, 1024]
    xnq = [[None] * 2 for _ in range(2)]   # [hl][quad] -> [128,4,2,8,128]
    wvt = [None] * 2                       # [hl] -> [128, 4, 2, 1024]

    def load_a(half):
        t = pool_a.tile([P, 2, 2, 8, P], F8, tag="a", name=f"a{half}")
        nc.sync.dma_start(out=t, in_=a_d[:, 2 * half:2 * half + 2, :, :, :])
        at[half] = t

    def load_xq(hl, qb):
        t = pool_xq.tile([P, 4, 2, IB], F8, tag="xq", name=f"xq{hl}_{qb}")
        nc.sync.dma_start(out=t, in_=xq_d[hl, qb, :, :, :, :])
        xqt[hl][qb] = t

    def load_xk(kh):
        t = pool_xk.tile([P, 4, 2, N // 2], F8, tag="xk", name=f"xk{kh}")
        nc.sync.dma_start(out=t, in_=xk_d[:, :, :,
                                          kh * (N // 2):(kh + 1) * (N // 2)])
        xkt[kh] = t

    def load_xn(hl, q):
        t = pool_xn.tile([P, 4, 2, 8, P], F8, tag="xn", name=f"xn{hl}_{q}")
        nc.sync.dma_start(out=t, in_=xn_d[hl, q, :, :, :, :, :])
        xnq[hl][q] = t

    def load_wv(hl):
        t = pool_wv.tile([P, 4, 2, NQ], F8, tag="wv", name=f"wv{hl}")
        nc.sync.dma_start(out=t, in_=wv_d[hl, :, :, :, :])
        wvt[hl] = t

    load_a(0)
    load_xq(0, 0)
    load_a(1)
    load_xq(1, 0)
    for qb in range(1, 4):
        load_xq(0, qb)
        load_xq(1, qb)
    load_xk(0)
    maskt = pool_mask.tile([P, 2, IBQ], BF16, tag="mask", name="maskt")
    nc.sync.dma_start(out=maskt, in_=mask_d[:, :, :])
    load_xn(0, 0)
    load_xn(1, 0)
    load_xk(1)
    load_wv(0)
    load_wv(1)
    load_xn(0, 1)
    load_xn(1, 1)
    ones = pool_one.tile([P, 2, 1], F8, tag="one", name="ones")
    nc.vector.memset(ones, 16.0)
    dummy = pool_one.tile([P, 2, IB], F8, tag="one", name="dummy")
    nc.vector.memset(dummy, 0.0)

    # T^T tiles per (qb, dp): [128, 2, 256] (dim1 = d2-tile parity)
    tht = [[pool_t.tile([P, 2, IB], F8, tag="tq", name=f"th{qb}_{dp}")
            for dp in range(4)] for qb in range(4)]

    with (
        tc.tile_pool(name="psA", bufs=7, space="PSUM") as ps_a,
        tc.tile_pool(name="psl", bufs=1, space="PSUM") as ps_l,
    ):
        probs = [None] * NS   # per slot: (ph quads, pl quads)
        rts = [None] * NS
        uqs = [None] * NS     # per slot: (uh quads, ul quads) by dt-half

        warm = [0]

        def warmup(n):
            # dependency-free full-shape DoubleRow matmuls keep the PE
            # p-state ramp alive while prologue DMAs land; output unread
            psd = ps_a.tile([P, 2, IB], F32, tag="ps", name=f"wu{warm[0]}")
            warm[0] += 1
            for i in range(n):
                _mm(nc, psd[:, 0, :], dummy[:, :, :IBQ], dummy,
                    start=(i == 0), stop=(i == n - 1))

        def emit_t(qb):
            # one full-bank window per d2-pair: halves are the two d2 tiles
            for dp in range(4):
                psw = ps_a.tile([P, 2, IB], F32, tag="ps", name=f"pt{qb}_{dp}")
                # x-lo contraction only over the first half of d1 (the
                # dropped Ah*xl tail contributes ~0.5% which the error
                # budget absorbs)
                k = 0
                for half in range(2):
                    d2t = 2 * dp + half
                    for hx in (0, 1):
                        for pp in range(4 if hx == 0 else 2):
                            _mm(nc, psw[:, half, :],
                                at[pp // 2][:, pp % 2, :, d2t, :],
                                xqt[hx][qb][:, pp, :, :],
                                start=(k == 0), stop=(k == 11))
                            k += 1
                nc.scalar.activation(tht[qb][dp], psw,
                                     mybir.ActivationFunctionType.Copy)

        def gen_scores(s):
            trips = _trips(s)
            qb, sh = s // 2, s % 2
            nw = (trips + 3) // 4
            ph_t = [pool_p.tile([P, 4, IBQ], F8, tag="probs",
                                name=f"ph{s}_{w}") for w in range(nw)]
            pl_t = [pool_p.tile([P, 4, IBQ], F8, tag="probs",
                                name=f"pl{s}_{w}") for w in range(nw)]
            probs[s] = (ph_t, pl_t)
            for w in range(nw):
                njt = min(4, trips - 4 * w)
                psw = ps_a.tile([P, 2, IB], F32, tag="ps", name=f"pss{s}_{w}")
                k = 0
                last = 8 * njt - 1
                for q in range(njt):
                    jt = 4 * w + q
                    for dp in range(4):
                        _mm(nc, psw[:, q // 2, (q % 2) * IBQ:(q % 2 + 1) * IBQ],
                            xkt[jt // 8][:, dp, :,
                                         (jt % 8) * P:(jt % 8 + 1) * P],
                            tht[qb][dp][:, :, sh * IBQ:(sh + 1) * IBQ],
                            start=(k == 0), stop=(k == last))
                        k += 1
                if w == nw - 1:
                    # mask the last two key tiles (the slot diagonal)
                    if njt == 2:
                        nc.vector.tensor_add(psw[:, 0, :], psw[:, 0, :],
                                             maskt)
                    else:
                        nc.vector.tensor_add(psw[:, 1, :], psw[:, 1, :],
                                             maskt)
                e = pool_e.tile([P, 2, IB], F32, tag="e32", name=f"e{s}_{w}")
                if njt == 4:
                    nc.scalar.activation(e, psw,
                                         mybir.ActivationFunctionType.Exp,
                                         scale=SCALE)
                    nc.vector.tensor_copy(ph_t[w], e)
                    nc.gpsimd.tensor_sub(pl_t[w], e, ph_t[w])
                else:
                    nc.scalar.activation(e[:, 0, :], psw[:, 0, :],
                                         mybir.ActivationFunctionType.Exp,
                                         scale=SCALE)
                    nc.vector.tensor_copy(ph_t[w][:, :2, :], e[:, 0, :])
                    nc.gpsimd.tensor_sub(pl_t[w][:, :2, :], e[:, 0, :],
                                         ph_t[w][:, :2, :])
                yield

        def gen_u(s):
            pairs = _trips(s) // 2
            ph_t, pl_t = probs[s]
            uh_t = [pool_u.tile([P, 4, IBQ], F8, tag="uq",
                                name=f"uh{s}_{h}") for h in range(2)]
            ul_t = [pool_u.tile([P, 4, IBQ], F8, tag="uq",
                                name=f"ul{s}_{h}") for h in range(2)]
            uqs[s] = (uh_t, ul_t)
            for h2 in range(2):
                psw = ps_a.tile([P, 2, IB], F32, tag="ps", name=f"psu{s}_{h2}")
                k = 0
                last = 12 * pairs - 1
                for q in range(4):
                    dt = 4 * h2 + q
                    for hx, hp in ((0, 0), (1, 0), (0, 1)):
                        pt_ = ph_t if hp == 0 else pl_t
                        for j in range(pairs):
                            _mm(nc, psw[:, q // 2, (q % 2) * IBQ:(q % 2 + 1) * IBQ],
                                xnq[hx][j // 4][:, j % 4, :, dt, :],
                                pt_[j // 2][:, 2 * (j % 2):2 * (j % 2) + 2, :],
                                start=(k == 0), stop=(k == last))
                            k += 1
                nc.scalar.activation(uh_t[h2], psw,
                                     mybir.ActivationFunctionType.Copy)
                nc.vector.tensor_sub(ul_t[h2], psw, uh_t[h2])
                yield
            # softmax denominator (x16): 1-col DoubleRow matmuls
            psl = ps_l.tile([P, 1], F32, tag="psl", name=f"psl{s}")
            k = 0
            last = 2 * pairs - 1
            for j in range(pairs):
                for t in (ph_t, pl_t):
                    _mm(nc, psl,
                        t[j // 2][:, 2 * (j % 2):2 * (j % 2) + 2, :], ones,
                        start=(k == 0), stop=(k == last))
                    k += 1
            rt = pool_rt.tile([P, 1], F32, tag="rt", name=f"rt{s}")
            nc.vector.reciprocal(rt, psl)
            rts[s] = rt
            yield

        def gen_out(s):
            uh_t, ul_t = uqs[s]
            ob_t = pool_ob.tile([P, D], BF16, tag="ob", name=f"ob{s}")
            for w in range(2):
                psw = ps_a.tile([P, 2, IB], F32, tag="ps", name=f"psf{s}_{w}")
                k = 0
                for half in range(2):
                    ob = 2 * w + half
                    for hu, hw in ((0, 0), (0, 1), (1, 0)):
                        ut = uh_t if hu == 0 else ul_t
                        for dp in range(4):
                            _mm(nc, psw[:, half, :],
                                ut[dp // 2][:, 2 * (dp % 2):2 * (dp % 2) + 2, :],
                                wvt[hw][:, dp, :, ob * IB:(ob + 1) * IB],
                                start=(k == 0), stop=(k == 23))
                            k += 1
                nc.vector.tensor_scalar_mul(ob_t[:, w * 2 * IB:(w + 1) * 2 * IB],
                                            psw, rts[s])
                if s == NS - 1 and w == 1:
                    nc.sync.dma_start(out=out_d[s * IBQ:(s + 1) * IBQ,
                                                :2 * IB],
                                      in_=ob_t[:, :2 * IB])
                    nc.sync.dma_start(out=out_d[s * IBQ:(s + 1) * IBQ,
                                                2 * IB:],
                                      in_=ob_t[:, 2 * IB:])
                yield
            if s != NS - 1:
                nc.sync.dma_start(out=out_d[s * IBQ:(s + 1) * IBQ, :],
                                  in_=ob_t)
            yield

        def rr(*gens):
            live = list(gens)
            while live:
                nxt = []
                for g in live:
                    try:
                        next(g)
                        nxt.append(g)
                    except StopIteration:
                        pass
                live = nxt

        warmup(44)
        emit_t(0)
        warmup(10)
        emit_t(1)
        emit_t(2)
        emit_t(3)
        rr(gen_scores(0))
        rr(gen_scores(1))
        rr(gen_scores(2), gen_u(0))
        rr(gen_scores(3), gen_u(1))
        rr(gen_scores(4), gen_u(2), gen_out(0))
        rr(gen_scores(5), gen_u(3), gen_out(1))
        rr(gen_scores(6), gen_u(4), gen_out(2))
        rr(gen_scores(7), gen_u(5), gen_out(3))
        rr(gen_u(6), gen_out(4))
        rr(gen_u(7), gen_out(5))
        rr(gen_out(6))
        rr(gen_out(7))


def _build_nc():
    nc = bacc.Bacc(None, target_bir_lowering=False)
    a_d = nc.declare_dram_parameter("a8", [P, 4, 2, 8, P], F8,
                                    isOutput=False)
    xq_d = nc.declare_dram_parameter("xq8", [2, 4, P, 4, 2, IB], F8,
                                     isOutput=False)
    xk_d = nc.declare_dram_parameter("xk8", [P, 4, 2, N], F8,
                                     isOutput=False)
    xn_d = nc.declare_dram_parameter("xn8", [2, 2, P, 4, 2, 8, P], F8,
                                    isOutput=False)
    wv_d = nc.declare_dram_parameter("wv8", [2, P, 4, 2, NQ], F8,
                                     isOutput=False)
    mask_d = nc.declare_dram_parameter("masks", [P, 2, IBQ], BF16,
                                       isOutput=False)
    out_d = nc.declare_dram_parameter("out_p", [NQ, D], BF16, isOutput=True)
    dram = (a_d, xq_d, xk_d, xn_d, wv_d, mask_d, out_d)

    with tile.TileContext(nc) as tc:
        with ExitStack() as ctx:
            _build_body(nc, tc, ctx, dram)
    nc.finalize()
    return nc


def _split8(a):
    h = a.astype(E4NP)
    l = (a - h.astype(np.float32)).astype(E4NP)
    return h, l


def _make_in_maps(x, W_q, W_k, W_v):
    wq = np.asarray(W_q, np.float32)
    wk = np.asarray(W_k, np.float32)
    wv = np.asarray(W_v, np.float32)
    A32 = (wq.T.astype(np.float64) @ wk.astype(np.float64) * 32.0).astype(np.float32)
    Wvt32 = np.ascontiguousarray(wv.T) * 32.0

    # a8: [2, 128, 4 pp, 2, 8, 128]
    def lhs_pack(m):       # [d1, d2] -> [128, 4, 2, 8, 128]
        return m.reshape(4, 2, P, 8, P).transpose(2, 0, 1, 3, 4)
    ah, _ = _split8(A32)
    a8 = np.ascontiguousarray(lhs_pack(ah))
    # wv8: [2, 128, 4 dp, 2, 1024]
    def rhs_pack(m):       # [d, o] -> [128, 4, 2, 1024]
        return m.reshape(4, 2, P, NQ).transpose(2, 0, 1, 3)
    wh, wl = _split8(Wvt32)
    wv8 = np.ascontiguousarray(np.stack([rhs_pack(wh), rhs_pack(wl)]))

    masks = [None, None]
    for h in range(2):
        mk = _build_masks(h).astype(BFNP)       # [2, 128, 128]
        masks[h] = np.ascontiguousarray(mk.transpose(1, 0, 2))
    qrows = [_qrows(0), _qrows(1)]

    in_maps = []
    per_batch = {}
    for b in range(B):
        xb = np.asarray(x[b], np.float32)
        xh, xl = _split8(xb)                    # [2048, 1024] e4m3
        xh32 = xh.astype(np.float32)
        xl32 = xl.astype(np.float32)
        # xk8: [2, 128, 4 dp, 2, 2048]: from x^T [1024 d, 2048 k]
        def xk_pack(m32):
            return m32.T.reshape(4, 2, P, N).transpose(2, 0, 1, 3)
        xk8 = np.ascontiguousarray(xk_pack(xh32)).astype(E4NP)
        # xn8: [2, 2 quad, 128, 4 kp, 2, 8 dt, 128]: from x/2 [2048, 1024]
        def xn_pack(m32):
            a = (m32 * 0.5).reshape(2, 4, 2, P, 8, P)
            return a.transpose(0, 3, 1, 2, 4, 5)
        xn8 = np.ascontiguousarray(np.stack(
            [xn_pack(xh32), xn_pack(xl32)])).astype(E4NP)
        per_batch[b] = (xh32, xl32, xk8, xn8)

    for c in range(NCORES):
        b, h = c // 2, c % 2
        xh32, xl32, xk8, xn8 = per_batch[b]
        # xq8: [2, 4 qb, 128, 4 pp, 2, 256]: x^T[:, qrows]
        def xq_pack(m32):
            # [1024 d1, 1024 q] -> [4 qb, 128, 4 pp, 2, 256]
            a = m32.T[:, qrows[h]].reshape(4, 2, P, 4, IB)
            return a.transpose(3, 2, 0, 1, 4)
        xq8 = np.ascontiguousarray(np.stack(
            [xq_pack(xh32), xq_pack(xl32)])).astype(E4NP)
        in_maps.append({
            "a8": a8, "xq8": xq8, "xk8": xk8, "xn8": xn8,
            "wv8": wv8, "masks": masks[h],
        })
    return in_maps


def kernel(x, W_q, W_k, W_v):
    global _CACHED_NC, LAST_RESULT, LAST_IN_MAPS
    x = np.asarray(x, dtype=np.float32)
    if _CACHED_NC is None:
        _CACHED_NC = _build_nc()
    nc = _CACHED_NC

    in_maps = _make_in_maps(x, W_q, W_k, W_v)
    LAST_IN_MAPS = in_maps
    try:
        res = run_bass_kernel_spmd(nc, in_maps, list(range(NCORES)))
    except Exception:
        # transient NRT_EXEC_UNIT_UNRECOVERABLE wedges clear on retry
        import time as _time
        _time.sleep(5)
        res = run_bass_kernel_spmd(nc, in_maps, list(range(NCORES)))
    LAST_RESULT = res

    qrows = [_qrows(0), _qrows(1)]
    out = np.empty((B, N, D), np.float32)
    for c in range(NCORES):
        b, h = c // 2, c % 2
        out[b, qrows[h], :] = np.asarray(res.results[c]["out_p"],
                                         dtype=np.float32)
    return out
